# revision 28
# baseline (speedup 1.0000x reference)
"""Trainium2 Bass kernel for nn_EmbeddingGATHead (gnn_message_passing).

Sharding (8 cores), v2 (bf16 + pipelined):
  - Pooling: node-sharded. Core r owns graph nodes 24r..24r+23; streams its
    25 MB feature slice [2048, 24, 128] (split across two engine DMA queues)
    and avg-pools -> pool_sum [128, 16kc, 24].
  - Pool AllGather is chunked into 4 channel groups (bf16) so the collective
    + layer-1 projection matmuls overlap the feature streaming. A tiny
    warmup AllGather absorbs the first-collective cold cost.
  - GAT projections: column-sharded by (proj, head), bf16 weights
    (2.1 MB/layer/core). AllToAll (bf16) re-shards to node-parallel.
  - Attention per core on its 24 nodes (4 cliques of 6): per-head z/lrelu in
    bf16, all 4 heads' scores accumulated into one [4, 144] PSUM tile via
    zero-padded att columns, batched softmax, per-head alpha-broadcast
    matmul + weighted sum.
  - Final: residual + BN-scale folded per node, transpose + per-core
    permutation matmul places each node into its image column, one 256 KB
    AllReduce produces the [32, 2048] output everywhere.
"""
import numpy as np

B, P, C, HWF = 32, 6, 2048, 128
N = B * P            # 192
M = 8                # cores
NB = N // M          # 24 nodes/core
GB = NB // P         # 4 cliques/core
HEADS, DHEAD, LAYERS = 4, 512, 2
KCH = C // 128       # 16 contraction chunks
DC = DHEAD // 128    # 4 dhead chunks
NG = 4               # pool AllGather channel groups
KPG = KCH // NG      # kc chunks per group
GC = C // NG         # channels per group
GPP = GB * P * P     # 144 pair slots per head

_NC_CACHE = {}


def _install_drain_patch():
    """This compiler build lowers Drain to a CTRL opcode with no sync-wait
    struct; re-emit the final drain's aggregated sem waits as standalone
    wait instructions on the sync engine."""
    import bass_rust
    from concourse.vector_clock import ScopedClock
    from concourse import tile as _tile

    if getattr(_tile.TileContext, "_dab_patched", False):
        return

    def _patched_dab(self, tick_clock, wait_clock):
        nc = self.nc
        drain_inst = nc.sync.drain()
        wait_clock.add_sem_waits(
            drain_inst.ins, ScopedClock({None: tick_clock.global_clock})
        )
        si = drain_inst.ins.sync_info
        waits = list(si.on_wait) if si and si.on_wait else []
        if waits:
            si.on_wait = []
            for w in waits:
                sem = bass_rust.SemaphoreHandle(w.ant_name, w.id)
                nc.sync.wait_ge(sem, w.wait_value)
        nc.all_engine_barrier()
        popped = nc._tile_sem_poison_stack.pop()
        assert popped is self._sem_poison
        nc.clear_and_free_semaphores(list(self.sems.allocated().values()))
        nc.all_engine_barrier()

    _tile.TileContext._drain_and_barrier = _patched_dab
    _tile.TileContext._dab_patched = True


def _split_sync_waits(nc, max_waits=1):
    """This walrus build rejects instructions carrying more than one sync
    wait; hoist extras into standalone EventSemaphore waits just before the
    instruction on the same engine stream."""
    import concourse.mybir as mybir
    import bass_rust

    n = 0
    for fn in nc.m.functions:
        for bb in fn.blocks:
            insts = list(bb.instructions)
            out = []
            changed = False
            for inst in insts:
                si = inst.sync_info
                waits = list(si.on_wait) if si and si.on_wait else []
                if len(waits) > max_waits:
                    si.on_wait = waits[:max_waits]
                    for w in waits[max_waits:]:
                        n += 1
                        wi = mybir.InstEventSemaphore(
                            name=f"WSPLIT-{n}", ins=[], outs=[]
                        )
                        wi.engine = inst.engine
                        wi.sync_info = bass_rust.SyncInfo(on_wait=[w], on_update=[])
                        out.append(wi)
                    changed = True
                out.append(inst)
            if changed:
                bb.instructions = out
    return nc


def _build():
    import concourse.bass as bass
    import concourse.mybir as mybir
    from concourse import tile

    _install_drain_patch()
    dt = mybir.dt.float32
    bt = mybir.dt.bfloat16
    ft16 = mybir.dt.float16
    AF = mybir.ActivationFunctionType
    ALU = mybir.AluOpType
    AX = mybir.AxisListType
    RG = [list(range(M))]

    nc = bass.Bass(num_devices=M)

    ht = None  # set below
    featT = nc.declare_dram_parameter("featT", [C, NB, HWF], mybir.dt.float16, isOutput=False)
    wsl = nc.declare_dram_parameter("wsl", [LAYERS, C, DHEAD], bt, isOutput=False)
    attp = nc.declare_dram_parameter(
        "attp", [LAYERS * HEADS * DC, 128, HEADS], bt, isOutput=False
    )
    adjf = nc.declare_dram_parameter("adjf", [HEADS, GPP], dt, isOutput=False)
    bnsc = nc.declare_dram_parameter("bnsc", [KCH, 2, 128], dt, isOutput=False)
    identb = nc.declare_dram_parameter("identb", [128, 128], bt, isOutput=False)
    permT = nc.declare_dram_parameter("permT", [NB, B], bt, isOutput=False)
    sel4 = nc.declare_dram_parameter("sel4", [HEADS, HEADS * 128], bt, isOutput=False)
    out_ext = nc.declare_dram_parameter("out", [B, C], dt, isOutput=True)

    with tile.TileContext(nc) as tc:
        with (
            tc.tile_pool(name="dram", bufs=1, space="DRAM") as dram,
            tc.tile_pool(name="consts", bufs=1) as consts,
            tc.tile_pool(name="wpool", bufs=1) as wpool,
            tc.tile_pool(name="fpool", bufs=6) as fpool,
            tc.tile_pool(name="ppool", bufs=1) as ppool,
            tc.tile_pool(name="rpool", bufs=2) as rpool,
            tc.tile_pool(name="apool", bufs=2) as apool,
            tc.tile_pool(name="zpool", bufs=4) as zpool,
            tc.tile_pool(name="spool", bufs=2) as spool,
            tc.tile_pool(name="opool", bufs=1) as opool,
            tc.tile_pool(name="mmps", bufs=1, space="PSUM") as mmps,
            tc.tile_pool(name="sps", bufs=1, space="PSUM") as sps,
            tc.tile_pool(name="sps2", bufs=2, space="PSUM") as sps2,
        ):
            # ---------------- internal DRAM ----------------
            warm_in = dram.tile([1, B], dt)
            warm_out = dram.tile([M, B], dt, addr_space="Shared")
            ag_in = [dram.tile([C // 2, NB], bt, name=f"agi{h}", tag=f"agi{h}") for h in range(2)]
            ag_out = [dram.tile([M, C // 2, NB], bt, name=f"ago{h}", tag=f"ago{h}", addr_space="Shared") for h in range(2)]
            a2a_in = [[dram.tile([M, DHEAD // 2, NB], bt, name=f"a2ai{l}{hf}", tag=f"a2ai{l}{hf}") for hf in range(2)] for l in range(LAYERS)]
            a2a_out = [[dram.tile([M, DHEAD // 2, NB], bt, name=f"a2ao{l}{hf}", tag=f"a2ao{l}{hf}") for hf in range(2)] for l in range(LAYERS)]
            agx_in = [dram.tile([C // 2, NB], bt, name=f"agxi{h}", tag=f"agxi{h}") for h in range(2)]
            agx_out = [dram.tile([M, C // 2, NB], bt, name=f"agxo{h}", tag=f"agxo{h}", addr_space="Shared") for h in range(2)]
            ar_in = [dram.tile([B, C // 2], dt, name=f"ari{h}", tag=f"ari{h}") for h in range(2)]
            ar_out = [dram.tile([B, C // 2], dt, name=f"aro{h}", tag=f"aro{h}", addr_space="Shared") for h in range(2)]

            # ---------------- warmup collective (absorbs ncfw cold start) --
            wt = consts.tile([1, B], dt)
            nc.vector.memset(wt[:], 0.0)
            nc.sync.dma_start(warm_in[:], wt[:])
            nc.gpsimd.collective_compute(
                "AllGather", ALU.bypass, replica_groups=RG,
                ins=[warm_in.opt()], outs=[warm_out.opt()],
            )

            # ---------------- constants ----------------
            att_sb = consts.tile([128, LAYERS * HEADS * DC, HEADS], bt)
            nc.scalar.dma_start(att_sb[:], attp.rearrange("x k m -> k x m"))
            adjf_sb = consts.tile([HEADS, GPP], dt)
            nc.scalar.dma_start(adjf_sb[:], adjf[:])
            bnsc_sb = consts.tile([128, KCH, 2], dt)
            nc.scalar.dma_start(bnsc_sb[:], bnsc.rearrange("c t d -> d c t"))
            ident_sb = consts.tile([128, 128], bt)
            nc.scalar.dma_start(ident_sb[:], identb[:])
            perm_sb = consts.tile([NB, B], bt)
            nc.scalar.dma_start(perm_sb[:], permT[:])
            sel_sb = consts.tile([HEADS, HEADS * 128], bt)
            nc.scalar.dma_start(sel_sb[:], sel4[:])

            # ---------------- weights (bf16; loaded after feature stream) --
            w_sb = [wpool.tile([128, KCH, DHEAD], bt, name=f"w{l}", tag=f"w{l}") for l in range(LAYERS)]

            # ---------------- pooling (features on 2 DMA queues) -----------
            pool_sum = ppool.tile([128, KCH, NB], dt)
            poolb = ppool.tile([128, KCH, NB], bt)
            fview = featT.rearrange("(kc k) n w -> kc k n w", k=128)
            rts = []
            for kc in range(KCH):
                ft = fpool.tile([128, NB, HWF], ft16, tag="ft")
                eng = nc.sync if kc % 2 == 0 else nc.scalar
                eng.dma_start(ft[:], fview[kc])
                nc.vector.reduce_sum(pool_sum[:, kc, :], ft[:], axis=AX.X)
                if kc % 8 == 7:
                    hf = kc // 8
                    hs = slice(hf * 8, (hf + 1) * 8)
                    nc.vector.tensor_scalar_mul(
                        poolb[:, hs, :], pool_sum[:, hs, :], 1.0 / HWF
                    )
                    nc.gpsimd.dma_start(
                        ag_in[hf].rearrange("(kc k) n -> k kc n", k=128),
                        poolb[:, hs, :],
                    )
                    nc.gpsimd.collective_compute(
                        "AllGather", ALU.bypass, replica_groups=RG,
                        ins=[ag_in[hf].opt()], outs=[ag_out[hf].opt()],
                    )
            for l in range(LAYERS):
                nc.scalar.dma_start(
                    w_sb[l][:], wsl[l].rearrange("(kc k) m -> k kc m", k=128)
                )

            x2_all = None
            for l in range(LAYERS):
                # ---- projections: xl^T/xr^T [512, 192] = W^T @ x^T (bf16) --
                rt = rpool.tile([128, KCH, N], bt, tag=f"rt{l}")
                srcs = ag_out if l == 0 else agx_out
                for hf in range(2):
                    hv = srcs[hf].rearrange("r (kc k) n -> kc k r n", k=128)
                    for kk in range(8):
                        kc = hf * 8 + kk
                        eng = nc.sync if kc % 2 == 0 else nc.scalar
                        eng.dma_start(
                            rt[:, kc, :].rearrange("p (r n) -> p r n", r=M),
                            hv[kk],
                        )
                pss = rpool.tile([128, DC, N], bt, tag=f"pss{l}")
                for half in range(2):
                    ps = [mmps.tile([128, N], dt, tag=f"mm{j}", name=f"mm{l}{half}{j}") for j in range(2)]
                    for j in range(2):
                        dc = half * 2 + j
                        for kc in range(KCH):
                            nc.tensor.matmul(
                                ps[j][:],
                                w_sb[l][:, kc, dc * 128:(dc + 1) * 128],
                                rt[:, kc, :],
                                start=(kc == 0),
                                stop=(kc == KCH - 1),
                            )
                    for j in range(2):
                        nc.vector.tensor_copy(pss[:, half * 2 + j, :], ps[j][:])
                    # ship this dc-half while the other half's matmuls run
                    hv2 = a2a_in[l][half].rearrange("s (dc d) n -> dc d s n", d=128)
                    for j in range(2):
                        nc.sync.dma_start(
                            hv2[j],
                            pss[:, half * 2 + j, :].rearrange("p (r n) -> p r n", r=M),
                        )
                    nc.gpsimd.collective_compute(
                        "AllToAll", ALU.bypass, replica_groups=RG,
                        ins=[a2a_in[l][half].opt()], outs=[a2a_out[l][half].opt()],
                    )
                # ---- local attention on my 24 nodes ----
                xt = apool.tile([128, 2 * HEADS, DC, NB], bt, tag=f"xt{l}")
                for s in range(2 * HEADS):
                    eng = nc.sync if s % 2 == 0 else nc.scalar
                    for hf in range(2):
                        eng.dma_start(
                            xt[:, s, hf * 2:(hf + 1) * 2],
                            a2a_out[l][hf][s].rearrange("(dc d) n -> d dc n", d=128),
                        )
                s4 = sps.tile([HEADS, GPP], dt, tag="s4", name=f"s4{l}")
                xlv = xt[:, 0:HEADS].rearrange(
                    "p h dc (g i) -> p h (dc g) i", g=GB
                )[:, :, :, None, :].to_broadcast([128, HEADS, DC * GB, P, P])
                xrv = xt[:, HEADS:2 * HEADS].rearrange(
                    "p h dc (g i) -> p h (dc g) i", g=GB
                )[:, :, :, :, None].to_broadcast([128, HEADS, DC * GB, P, P])
                z2 = zpool.tile([128, HEADS, DC * GB, P, P], bt, tag="z")
                nc.vector.tensor_tensor(z2[:], xrv, xlv, ALU.add)
                zf = z2.rearrange("p h dcg i j -> p (h dcg i j)")
                lz = zpool.tile([128, HEADS * DC * GPP], bt, tag="lz")
                # lrelu(z) = max(0.2*z, z)
                nc.vector.scalar_tensor_tensor(
                    lz[:], zf, 0.2, zf, ALU.mult, ALU.max
                )
                for h in range(HEADS):
                    for dc in range(DC):
                        off = (h * KCH + dc * GB) * P * P
                        nc.tensor.matmul(
                            s4[:],
                            att_sb[:, (l * HEADS + h) * DC + dc, :],
                            lz[:, off:off + GPP],
                            start=(h == 0 and dc == 0),
                            stop=(h == HEADS - 1 and dc == DC - 1),
                        )
                # masked softmax over the 6 sources (exp without max-shift)
                e4 = spool.tile([HEADS, GPP], dt, tag=f"e4{l}", name=f"e4{l}")
                nc.scalar.activation(e4[:], s4[:], AF.Exp)
                em4 = spool.tile([HEADS, GPP], dt, tag=f"em{l}", name=f"em{l}")
                nc.vector.tensor_tensor(em4[:], e4[:], adjf_sb[:], ALU.mult)
                ssum = spool.tile([HEADS, GB * P], dt, tag=f"ss{l}", name=f"ss{l}")
                nc.vector.reduce_sum(
                    ssum[:], em4.rearrange("p (gi j) -> p gi j", j=P), axis=AX.X
                )
                rec = spool.tile([HEADS, GB * P], dt, tag=f"rc{l}", name=f"rc{l}")
                nc.vector.reciprocal(rec[:], ssum[:])
                alpha4 = spool.tile([HEADS, GPP], bt, tag=f"al{l}", name=f"al{l}")
                nc.vector.tensor_tensor(
                    alpha4.rearrange("p (gi j) -> p gi j", j=P),
                    em4.rearrange("p (gi j) -> p gi j", j=P),
                    rec[:, :, None].to_broadcast([HEADS, GB * P, P]),
                    ALU.mult,
                )
                # aggregation: out[i] = sum_j alpha[i,j] xl[j]
                out_all = opool.tile([128, HEADS, DC, NB], dt, tag=f"oa{l}")
                for h in range(HEADS):
                    ab_ps = sps.tile([128, GPP], dt, tag="ab", name=f"ab{l}{h}")
                    nc.tensor.matmul(
                        ab_ps[:], sel_sb[:, h * 128:(h + 1) * 128], alpha4[:],
                        start=True, stop=True,
                    )
                    ab = apool.tile([128, GPP], bt, tag="abs")
                    nc.vector.tensor_copy(ab[:], ab_ps[:])
                    ab5 = ab.rearrange("p (g i j) -> p g i j", g=GB, i=P)[
                        :, None, :, :, :
                    ].to_broadcast([128, DC, GB, P, P])
                    xl5 = xt[:, h].rearrange("p dc (g i) -> p dc g i", g=GB)[
                        :, :, :, None, :
                    ].to_broadcast([128, DC, GB, P, P])
                    prod = zpool.tile([128, DC, GB, P, P], bt, tag="pr")
                    nc.vector.tensor_tensor(prod[:], ab5, xl5, ALU.mult)
                    nc.vector.reduce_sum(
                        out_all[:, h],
                        prod.rearrange("p dc g i j -> p dc (g i) j"),
                        axis=AX.X,
                    )
                oflat = out_all.rearrange("p h dc n -> p (h dc n)")
                if l == 0:
                    HHN = 2 * DC * NB
                    for hf in range(2):
                        hsl = slice(hf * HHN, (hf + 1) * HHN)
                        t1 = opool.tile([128, HHN], dt, tag="t1")
                        nc.vector.tensor_scalar_min(t1[:], oflat[:, hsl], 0.0)
                        t2 = opool.tile([128, HHN], dt, tag="t2")
                        nc.scalar.activation(t2[:], t1[:], AF.Exp)
                        x1b = opool.tile([128, HHN], bt, tag="x1b")
                        # elu(x) = max(exp(min(x,0)) - 1, x)
                        nc.vector.scalar_tensor_tensor(
                            x1b[:], t2[:], -1.0, oflat[:, hsl], ALU.add, ALU.max
                        )
                        nc.sync.dma_start(
                            agx_in[hf].rearrange("(hd d) n -> d hd n", d=128),
                            x1b.rearrange("p (hd n) -> p hd n", n=NB),
                        )
                        nc.gpsimd.collective_compute(
                            "AllGather", ALU.bypass, replica_groups=RG,
                            ins=[agx_in[hf].opt()], outs=[agx_out[hf].opt()],
                        )
                else:
                    x2_all = opool.tile([128, KCH, NB], dt, tag="x2")
                    # residual: out + pool_sum/128
                    nc.vector.scalar_tensor_tensor(
                        x2_all.rearrange("p kc n -> p (kc n)"),
                        pool_sum.rearrange("p kc n -> p (kc n)"),
                        1.0 / HWF, oflat, ALU.mult, ALU.add,
                    )

            # ------- final: BN-scale, transpose, node->image perm, AllReduce
            arin = ppool.tile([B, KCH, 128], dt)
            for kc in range(KCH):
                x2s = rpool.tile([128, NB], bt, tag="x2s")
                nc.scalar.activation(
                    x2s[:], x2_all[:, kc, :], AF.Identity,
                    bias=bnsc_sb[:, kc, 1:2], scale=bnsc_sb[:, kc, 0:1],
                )
                tp = sps2.tile([NB, 128], bt, tag="tp")
                nc.tensor.transpose(tp[:], x2s[:], ident_sb[:])
                x2t = rpool.tile([NB, 128], bt, tag="x2t")
                nc.vector.tensor_copy(x2t[:], tp[:])
                arps = sps2.tile([B, 128], dt, tag="arps")
                nc.tensor.matmul(arps[:], perm_sb[:], x2t[:], start=True, stop=True)
                nc.scalar.copy(arin[:, kc, :], arps[:])
                if kc % 8 == 7:
                    hf = kc // 8
                    nc.sync.dma_start(
                        ar_in[hf].rearrange("b (kc d) -> b kc d", d=128),
                        arin[:, hf * 8:(hf + 1) * 8, :],
                    )
                    nc.gpsimd.collective_compute(
                        "AllReduce", ALU.add, replica_groups=RG,
                        ins=[ar_in[hf].opt()], outs=[ar_out[hf].opt()],
                    )
                    nc.sync.dma_start(
                        out_ext[:, hf * 1024:(hf + 1) * 1024], ar_out[hf][:]
                    )

    _split_sync_waits(nc)
    return nc


def _prep_inputs(features, img_num_ps, Wl, bl, Wr, br, att, gat_bias,
                 bn_gamma, bn_mean, bn_var):
    import ml_dtypes
    f32 = np.float32
    bf16 = ml_dtypes.bfloat16
    features = np.asarray(features, f32)
    inp = np.asarray(img_num_ps)
    Wl = np.asarray(Wl, f32)
    Wr = np.asarray(Wr, f32)
    att = np.asarray(att, f32)
    bn_gamma = np.asarray(bn_gamma, f32)
    bn_mean = np.asarray(bn_mean, f32)
    bn_var = np.asarray(bn_var, f32)

    parts = features.reshape(B, P, C, HWF).transpose(1, 0, 2, 3).reshape(N, C, HWF)
    # zero-padded per-head att columns: attp[(l,h,dc), k, m] = att[l,h,dc*128+k]
    # iff m == h else 0
    attp_np = np.zeros((LAYERS, HEADS, DC, 128, HEADS), f32)
    for l in range(LAYERS):
        for h in range(HEADS):
            attp_np[l, h, :, :, h] = att[l, h].reshape(DC, 128)
    attp_np = attp_np.reshape(LAYERS * HEADS * DC, 128, HEADS).astype(bf16)
    scale = bn_gamma / np.sqrt(bn_var + 1e-5)
    bnsc_np = np.stack(
        [(scale / P).reshape(KCH, 128),
         (-scale * bn_mean / P).reshape(KCH, 128)],
        axis=1,
    ).astype(f32)
    ident_np = np.eye(128, dtype=f32).astype(bf16)
    sel4_np = np.zeros((HEADS, HEADS * 128), f32)
    for h in range(HEADS):
        sel4_np[h, h * 128:(h + 1) * 128] = 1.0
    sel4_np = sel4_np.astype(bf16)

    in_maps = []
    for r in range(M):
        featT_r = np.ascontiguousarray(
            parts[r * NB:(r + 1) * NB].transpose(1, 0, 2)
        ).astype(np.float16)
        wsl_r = np.ascontiguousarray(
            (Wl if r < HEADS else Wr)[:, r % HEADS]
        ).astype(bf16)
        a = np.zeros((GB, P, P), f32)
        for gl in range(GB):
            v = np.arange(P) < inp[GB * r + gl]
            a[gl] = ((v[:, None] & v[None, :]) | np.eye(P, dtype=bool))
        adjf_r = np.tile(a.reshape(1, GB * P * P), (HEADS, 1)).astype(f32)
        permT_r = np.zeros((NB, B), f32)
        for n in range(NB):
            permT_r[n, (r * NB + n) % B] = 1.0
        in_maps.append({
            "featT": featT_r,
            "wsl": wsl_r,
            "attp": attp_np,
            "adjf": adjf_r,
            "bnsc": bnsc_np,
            "identb": ident_np,
            "permT": permT_r.astype(bf16),
            "sel4": sel4_np,
        })
    return in_maps


def _run(inputs, trace=False):
    from concourse.bass_utils import run_bass_kernel_spmd

    if "nc" not in _NC_CACHE:
        _NC_CACHE["nc"] = _build()
    nc = _NC_CACHE["nc"]
    in_maps = _prep_inputs(**inputs)
    res = run_bass_kernel_spmd(
        nc, in_maps, core_ids=list(range(M)), trace=trace
    )
    return res


def kernel(**inputs):
    res = _run(inputs, trace=False)
    return np.asarray(res.results[0]["out"], np.float32)


# revision 30
# speedup vs baseline: 1.0688x; 1.0688x over previous
"""Trainium2 Bass kernel for nn_EmbeddingGATHead (gnn_message_passing).

Sharding (8 cores), bf16/fp16 pipelined:
  - Pooling: node-sharded. Core r owns graph nodes 24r..24r+23; streams its
    12.6 MB fp16 feature slice [2048, 24, 128] on two engine DMA queues
    (features before weights to avoid head-of-line blocking) and avg-pools
    on the vector engine -> pool_sum [128, 16kc, 24].
  - A tiny warmup AllGather issued at t=0 absorbs the ~70us first-collective
    cold cost under the pooling phase.
  - Pool AllGather (bf16), then GAT projections column-sharded by
    (proj, head) with bf16 weights (2.1 MB/layer/core). Each layer's
    AllToAll is split into two dhead-halves so the first half's trigger
    latency hides under the second half's projection matmuls.
  - Attention per core on its 24 nodes (4 cliques of 6): single batched
    z/lrelu over all heads via a merged (dc g) axis, all 4 heads' scores
    accumulated into one [4, 144] PSUM tile via zero-padded att columns,
    batched softmax, per-head alpha-broadcast matmul + weighted sum.
  - Final: residual + BN-scale folded per node, transpose + per-core
    permutation matmul places each node into its image column, one 256 KB
    AllReduce produces the [32, 2048] output everywhere.
"""
import numpy as np

B, P, C, HWF = 32, 6, 2048, 128
N = B * P            # 192
M = 8                # cores
NB = N // M          # 24 nodes/core
GB = NB // P         # 4 cliques/core
HEADS, DHEAD, LAYERS = 4, 512, 2
KCH = C // 128       # 16 contraction chunks
DC = DHEAD // 128    # 4 dhead chunks
NG = 4               # pool AllGather channel groups
KPG = KCH // NG      # kc chunks per group
GC = C // NG         # channels per group
GPP = GB * P * P     # 144 pair slots per head

_NC_CACHE = {}


def _install_drain_patch():
    """This compiler build lowers Drain to a CTRL opcode with no sync-wait
    struct; re-emit the final drain's aggregated sem waits as standalone
    wait instructions on the sync engine."""
    import bass_rust
    from concourse.vector_clock import ScopedClock
    from concourse import tile as _tile

    if getattr(_tile.TileContext, "_dab_patched", False):
        return

    def _patched_dab(self, tick_clock, wait_clock):
        nc = self.nc
        drain_inst = nc.sync.drain()
        wait_clock.add_sem_waits(
            drain_inst.ins, ScopedClock({None: tick_clock.global_clock})
        )
        si = drain_inst.ins.sync_info
        waits = list(si.on_wait) if si and si.on_wait else []
        if waits:
            si.on_wait = []
            for w in waits:
                sem = bass_rust.SemaphoreHandle(w.ant_name, w.id)
                nc.sync.wait_ge(sem, w.wait_value)
        nc.all_engine_barrier()
        popped = nc._tile_sem_poison_stack.pop()
        assert popped is self._sem_poison
        nc.clear_and_free_semaphores(list(self.sems.allocated().values()))
        nc.all_engine_barrier()

    _tile.TileContext._drain_and_barrier = _patched_dab
    _tile.TileContext._dab_patched = True


def _split_sync_waits(nc, max_waits=1):
    """This walrus build rejects instructions carrying more than one sync
    wait; hoist extras into standalone EventSemaphore waits just before the
    instruction on the same engine stream."""
    import concourse.mybir as mybir
    import bass_rust

    n = 0
    for fn in nc.m.functions:
        for bb in fn.blocks:
            insts = list(bb.instructions)
            out = []
            changed = False
            for inst in insts:
                si = inst.sync_info
                waits = list(si.on_wait) if si and si.on_wait else []
                if len(waits) > max_waits:
                    si.on_wait = waits[:max_waits]
                    for w in waits[max_waits:]:
                        n += 1
                        wi = mybir.InstEventSemaphore(
                            name=f"WSPLIT-{n}", ins=[], outs=[]
                        )
                        wi.engine = inst.engine
                        wi.sync_info = bass_rust.SyncInfo(on_wait=[w], on_update=[])
                        out.append(wi)
                    changed = True
                out.append(inst)
            if changed:
                bb.instructions = out
    return nc


def _build():
    import concourse.bass as bass
    import concourse.mybir as mybir
    from concourse import tile

    _install_drain_patch()
    dt = mybir.dt.float32
    bt = mybir.dt.bfloat16
    ft16 = mybir.dt.float16
    AF = mybir.ActivationFunctionType
    ALU = mybir.AluOpType
    AX = mybir.AxisListType
    RG = [list(range(M))]

    nc = bass.Bass(num_devices=M)

    featT = nc.declare_dram_parameter("featT", [C, NB, HWF], mybir.dt.float16, isOutput=False)
    wsl = nc.declare_dram_parameter("wsl", [LAYERS, C, DHEAD], bt, isOutput=False)
    attp = nc.declare_dram_parameter(
        "attp", [LAYERS * HEADS * DC, 128, HEADS], bt, isOutput=False
    )
    adjf = nc.declare_dram_parameter("adjf", [HEADS, GPP], dt, isOutput=False)
    bnsc = nc.declare_dram_parameter("bnsc", [KCH, 2, 128], dt, isOutput=False)
    identb = nc.declare_dram_parameter("identb", [128, 128], bt, isOutput=False)
    permT = nc.declare_dram_parameter("permT", [NB, B], bt, isOutput=False)
    sel4 = nc.declare_dram_parameter("sel4", [HEADS, HEADS * 128], bt, isOutput=False)
    out_ext = nc.declare_dram_parameter("out", [B, C], dt, isOutput=True)

    with tile.TileContext(nc) as tc:
        with (
            tc.tile_pool(name="dram", bufs=1, space="DRAM") as dram,
            tc.tile_pool(name="consts", bufs=1) as consts,
            tc.tile_pool(name="wpool", bufs=1) as wpool,
            tc.tile_pool(name="fpool", bufs=6) as fpool,
            tc.tile_pool(name="ppool", bufs=1) as ppool,
            tc.tile_pool(name="rpool", bufs=2) as rpool,
            tc.tile_pool(name="apool", bufs=2) as apool,
            tc.tile_pool(name="zpool", bufs=4) as zpool,
            tc.tile_pool(name="spool", bufs=2) as spool,
            tc.tile_pool(name="opool", bufs=1) as opool,
            tc.tile_pool(name="mmps", bufs=1, space="PSUM") as mmps,
            tc.tile_pool(name="sps", bufs=1, space="PSUM") as sps,
            tc.tile_pool(name="sps2", bufs=2, space="PSUM") as sps2,
        ):
            # ---------------- internal DRAM ----------------
            warm_in = dram.tile([1, B], dt)
            warm_out = dram.tile([M, B], dt, addr_space="Shared")
            ag_in = dram.tile([C, NB], bt, name="agi", tag="agi")
            ag_out = dram.tile([M, C, NB], bt, name="ago", tag="ago", addr_space="Shared")
            a2a_in = [[dram.tile([M, DHEAD // 2, NB], bt, name=f"a2ai{l}{hf}", tag=f"a2ai{l}{hf}") for hf in range(2)] for l in range(LAYERS)]
            a2a_out = [[dram.tile([M, DHEAD // 2, NB], bt, name=f"a2ao{l}{hf}", tag=f"a2ao{l}{hf}") for hf in range(2)] for l in range(LAYERS)]
            agx_in = dram.tile([C, NB], bt, name="agxi", tag="agxi")
            agx_out = dram.tile([M, C, NB], bt, name="agxo", tag="agxo", addr_space="Shared")
            ar_in = dram.tile([B, C], dt, name="ari", tag="ari")
            ar_out = dram.tile([B, C], dt, name="aro", tag="aro", addr_space="Shared")

            # ---------------- warmup collective (absorbs ncfw cold start) --
            wt = consts.tile([1, B], dt)
            nc.vector.memset(wt[:], 0.0)
            nc.sync.dma_start(warm_in[:], wt[:])
            nc.gpsimd.collective_compute(
                "AllGather", ALU.bypass, replica_groups=RG,
                ins=[warm_in.opt()], outs=[warm_out.opt()],
            )

            # ---------------- constants ----------------
            att_sb = consts.tile([128, LAYERS * HEADS * DC, HEADS], bt)
            nc.scalar.dma_start(att_sb[:], attp.rearrange("x k m -> k x m"))
            adjf_sb = consts.tile([HEADS, GPP], dt)
            nc.scalar.dma_start(adjf_sb[:], adjf[:])
            bnsc_sb = consts.tile([128, KCH, 2], dt)
            nc.scalar.dma_start(bnsc_sb[:], bnsc.rearrange("c t d -> d c t"))
            ident_sb = consts.tile([128, 128], bt)
            nc.scalar.dma_start(ident_sb[:], identb[:])
            perm_sb = consts.tile([NB, B], bt)
            nc.scalar.dma_start(perm_sb[:], permT[:])
            sel_sb = consts.tile([HEADS, HEADS * 128], bt)
            nc.scalar.dma_start(sel_sb[:], sel4[:])

            # ---------------- weights (bf16; loaded after feature stream) --
            w_sb = [wpool.tile([128, KCH, DHEAD], bt, name=f"w{l}", tag=f"w{l}") for l in range(LAYERS)]

            # ---------------- pooling (features on 2 DMA queues) -----------
            pool_sum = ppool.tile([128, KCH, NB], dt)
            poolb = ppool.tile([128, KCH, NB], bt)
            fview = featT.rearrange("(kc k) n w -> kc k n w", k=128)
            rts = []
            for kc in range(KCH):
                ft = fpool.tile([128, NB, HWF], ft16, tag="ft")
                eng = nc.sync if kc % 2 == 0 else nc.scalar
                eng.dma_start(ft[:], fview[kc])
                nc.vector.reduce_sum(pool_sum[:, kc, :], ft[:], axis=AX.X)
            for l in range(LAYERS):
                nc.scalar.dma_start(
                    w_sb[l][:], wsl[l].rearrange("(kc k) m -> k kc m", k=128)
                )
            nc.vector.tensor_scalar_mul(
                poolb.rearrange("p kc n -> p (kc n)"),
                pool_sum.rearrange("p kc n -> p (kc n)"), 1.0 / HWF
            )
            nc.gpsimd.dma_start(
                ag_in.rearrange("(kc k) n -> k kc n", k=128), poolb[:]
            )
            nc.gpsimd.collective_compute(
                "AllGather", ALU.bypass, replica_groups=RG,
                ins=[ag_in.opt()], outs=[ag_out.opt()],
            )

            x2_all = None
            for l in range(LAYERS):
                # ---- projections: xl^T/xr^T [512, 192] = W^T @ x^T (bf16) --
                rt = rpool.tile([128, KCH, N], bt, tag=f"rt{l}")
                src_dram = ag_out if l == 0 else agx_out
                agxv = src_dram.rearrange("r (kc k) n -> kc k r n", k=128)
                for kc in range(KCH):
                    eng = nc.sync if kc % 2 == 0 else nc.scalar
                    eng.dma_start(
                        rt[:, kc, :].rearrange("p (r n) -> p r n", r=M),
                        agxv[kc],
                    )
                pss = rpool.tile([128, DC, N], bt, tag=f"pss{l}")
                for half in range(2):
                    ps = [mmps.tile([128, N], dt, tag=f"mm{j}", name=f"mm{l}{half}{j}") for j in range(2)]
                    for j in range(2):
                        dc = half * 2 + j
                        for kc in range(KCH):
                            nc.tensor.matmul(
                                ps[j][:],
                                w_sb[l][:, kc, dc * 128:(dc + 1) * 128],
                                rt[:, kc, :],
                                start=(kc == 0),
                                stop=(kc == KCH - 1),
                            )
                    for j in range(2):
                        nc.vector.tensor_copy(pss[:, half * 2 + j, :], ps[j][:])
                    # ship this dc-half while the other half's matmuls run
                    hv = a2a_in[l][half].rearrange("s (dc d) n -> dc d s n", d=128)
                    for j in range(2):
                        nc.sync.dma_start(
                            hv[j],
                            pss[:, half * 2 + j, :].rearrange("p (r n) -> p r n", r=M),
                        )
                    nc.gpsimd.collective_compute(
                        "AllToAll", ALU.bypass, replica_groups=RG,
                        ins=[a2a_in[l][half].opt()], outs=[a2a_out[l][half].opt()],
                    )
                # ---- local attention on my 24 nodes ----
                xt = apool.tile([128, 2 * HEADS, DC, NB], bt, tag=f"xt{l}")
                for s in range(2 * HEADS):
                    eng = nc.sync if s % 2 == 0 else nc.scalar
                    for hf in range(2):
                        eng.dma_start(
                            xt[:, s, hf * 2:(hf + 1) * 2],
                            a2a_out[l][hf][s].rearrange("(dc d) n -> d dc n", d=128),
                        )
                s4 = sps.tile([HEADS, GPP], dt, tag="s4", name=f"s4{l}")
                xlv = xt[:, 0:HEADS].rearrange(
                    "p h dc (g i) -> p h (dc g) i", g=GB
                )[:, :, :, None, :].to_broadcast([128, HEADS, DC * GB, P, P])
                xrv = xt[:, HEADS:2 * HEADS].rearrange(
                    "p h dc (g i) -> p h (dc g) i", g=GB
                )[:, :, :, :, None].to_broadcast([128, HEADS, DC * GB, P, P])
                z2 = zpool.tile([128, HEADS, DC * GB, P, P], bt, tag="z")
                nc.vector.tensor_tensor(z2[:], xrv, xlv, ALU.add)
                zf = z2.rearrange("p h dcg i j -> p (h dcg i j)")
                lz = zpool.tile([128, HEADS * DC * GPP], bt, tag="lz")
                # lrelu(z) = max(0.2*z, z)
                nc.vector.scalar_tensor_tensor(
                    lz[:], zf, 0.2, zf, ALU.mult, ALU.max
                )
                for h in range(HEADS):
                    for dc in range(DC):
                        off = (h * KCH + dc * GB) * P * P
                        nc.tensor.matmul(
                            s4[:],
                            att_sb[:, (l * HEADS + h) * DC + dc, :],
                            lz[:, off:off + GPP],
                            start=(h == 0 and dc == 0),
                            stop=(h == HEADS - 1 and dc == DC - 1),
                        )
                # masked softmax over the 6 sources (exp without max-shift)
                e4 = spool.tile([HEADS, GPP], dt, tag=f"e4{l}", name=f"e4{l}")
                nc.scalar.activation(e4[:], s4[:], AF.Exp)
                em4 = spool.tile([HEADS, GPP], dt, tag=f"em{l}", name=f"em{l}")
                nc.vector.tensor_tensor(em4[:], e4[:], adjf_sb[:], ALU.mult)
                ssum = spool.tile([HEADS, GB * P], dt, tag=f"ss{l}", name=f"ss{l}")
                nc.vector.reduce_sum(
                    ssum[:], em4.rearrange("p (gi j) -> p gi j", j=P), axis=AX.X
                )
                rec = spool.tile([HEADS, GB * P], dt, tag=f"rc{l}", name=f"rc{l}")
                nc.vector.reciprocal(rec[:], ssum[:])
                alpha4 = spool.tile([HEADS, GPP], bt, tag=f"al{l}", name=f"al{l}")
                nc.vector.tensor_tensor(
                    alpha4.rearrange("p (gi j) -> p gi j", j=P),
                    em4.rearrange("p (gi j) -> p gi j", j=P),
                    rec[:, :, None].to_broadcast([HEADS, GB * P, P]),
                    ALU.mult,
                )
                # aggregation: out[i] = sum_j alpha[i,j] xl[j]
                out_all = opool.tile([128, HEADS, DC, NB], dt, tag=f"oa{l}")
                for h in range(HEADS):
                    ab_ps = sps.tile([128, GPP], dt, tag="ab", name=f"ab{l}{h}")
                    nc.tensor.matmul(
                        ab_ps[:], sel_sb[:, h * 128:(h + 1) * 128], alpha4[:],
                        start=True, stop=True,
                    )
                    ab = apool.tile([128, GPP], bt, tag="abs")
                    nc.vector.tensor_copy(ab[:], ab_ps[:])
                    ab5 = ab.rearrange("p (g i j) -> p g i j", g=GB, i=P)[
                        :, None, :, :, :
                    ].to_broadcast([128, DC, GB, P, P])
                    xl5 = xt[:, h].rearrange("p dc (g i) -> p dc g i", g=GB)[
                        :, :, :, None, :
                    ].to_broadcast([128, DC, GB, P, P])
                    prod = zpool.tile([128, DC, GB, P, P], bt, tag="pr")
                    nc.vector.tensor_tensor(prod[:], ab5, xl5, ALU.mult)
                    nc.vector.reduce_sum(
                        out_all[:, h],
                        prod.rearrange("p dc g i j -> p dc (g i) j"),
                        axis=AX.X,
                    )
                oflat = out_all.rearrange("p h dc n -> p (h dc n)")
                if l == 0:
                    t1 = opool.tile([128, HEADS * DC * NB], dt, tag="t1")
                    nc.vector.tensor_scalar_min(t1[:], oflat, 0.0)
                    t2 = opool.tile([128, HEADS * DC * NB], dt, tag="t2")
                    nc.scalar.activation(t2[:], t1[:], AF.Exp)
                    x1b = opool.tile([128, HEADS * DC * NB], bt, tag="x1b")
                    # elu(x) = max(exp(min(x,0)) - 1, x)
                    nc.vector.scalar_tensor_tensor(
                        x1b[:], t2[:], -1.0, oflat, ALU.add, ALU.max
                    )
                    nc.sync.dma_start(
                        agx_in.rearrange("(hd d) n -> d hd n", d=128),
                        x1b.rearrange("p (hd n) -> p hd n", n=NB),
                    )
                    nc.gpsimd.collective_compute(
                        "AllGather", ALU.bypass, replica_groups=RG,
                        ins=[agx_in.opt()], outs=[agx_out.opt()],
                    )
                else:
                    x2_all = opool.tile([128, KCH, NB], dt, tag="x2")
                    # residual: out + pool_sum/128
                    nc.vector.scalar_tensor_tensor(
                        x2_all.rearrange("p kc n -> p (kc n)"),
                        pool_sum.rearrange("p kc n -> p (kc n)"),
                        1.0 / HWF, oflat, ALU.mult, ALU.add,
                    )

            # ------- final: BN-scale, transpose, node->image perm, AllReduce
            arin = ppool.tile([B, KCH, 128], dt)
            for kc in range(KCH):
                x2s = rpool.tile([128, NB], bt, tag="x2s")
                nc.scalar.activation(
                    x2s[:], x2_all[:, kc, :], AF.Identity,
                    bias=bnsc_sb[:, kc, 1:2], scale=bnsc_sb[:, kc, 0:1],
                )
                tp = sps2.tile([NB, 128], bt, tag="tp")
                nc.tensor.transpose(tp[:], x2s[:], ident_sb[:])
                x2t = rpool.tile([NB, 128], bt, tag="x2t")
                nc.vector.tensor_copy(x2t[:], tp[:])
                arps = sps2.tile([B, 128], dt, tag="arps")
                nc.tensor.matmul(arps[:], perm_sb[:], x2t[:], start=True, stop=True)
                nc.scalar.copy(arin[:, kc, :], arps[:])
            nc.sync.dma_start(
                ar_in.rearrange("b (kc d) -> b kc d", d=128), arin[:]
            )
            nc.gpsimd.collective_compute(
                "AllReduce", ALU.add, replica_groups=RG,
                ins=[ar_in.opt()], outs=[ar_out.opt()],
            )
            nc.sync.dma_start(out_ext[:], ar_out[:])

    _split_sync_waits(nc)
    return nc


def _prep_inputs(features, img_num_ps, Wl, bl, Wr, br, att, gat_bias,
                 bn_gamma, bn_mean, bn_var):
    import ml_dtypes
    f32 = np.float32
    bf16 = ml_dtypes.bfloat16
    features = np.asarray(features, f32)
    inp = np.asarray(img_num_ps)
    Wl = np.asarray(Wl, f32)
    Wr = np.asarray(Wr, f32)
    att = np.asarray(att, f32)
    bn_gamma = np.asarray(bn_gamma, f32)
    bn_mean = np.asarray(bn_mean, f32)
    bn_var = np.asarray(bn_var, f32)

    parts = features.reshape(B, P, C, HWF).transpose(1, 0, 2, 3).reshape(N, C, HWF)
    # zero-padded per-head att columns: attp[(l,h,dc), k, m] = att[l,h,dc*128+k]
    # iff m == h else 0
    attp_np = np.zeros((LAYERS, HEADS, DC, 128, HEADS), f32)
    for l in range(LAYERS):
        for h in range(HEADS):
            attp_np[l, h, :, :, h] = att[l, h].reshape(DC, 128)
    attp_np = attp_np.reshape(LAYERS * HEADS * DC, 128, HEADS).astype(bf16)
    scale = bn_gamma / np.sqrt(bn_var + 1e-5)
    bnsc_np = np.stack(
        [(scale / P).reshape(KCH, 128),
         (-scale * bn_mean / P).reshape(KCH, 128)],
        axis=1,
    ).astype(f32)
    ident_np = np.eye(128, dtype=f32).astype(bf16)
    sel4_np = np.zeros((HEADS, HEADS * 128), f32)
    for h in range(HEADS):
        sel4_np[h, h * 128:(h + 1) * 128] = 1.0
    sel4_np = sel4_np.astype(bf16)

    in_maps = []
    for r in range(M):
        featT_r = np.ascontiguousarray(
            parts[r * NB:(r + 1) * NB].transpose(1, 0, 2)
        ).astype(np.float16)
        wsl_r = np.ascontiguousarray(
            (Wl if r < HEADS else Wr)[:, r % HEADS]
        ).astype(bf16)
        a = np.zeros((GB, P, P), f32)
        for gl in range(GB):
            v = np.arange(P) < inp[GB * r + gl]
            a[gl] = ((v[:, None] & v[None, :]) | np.eye(P, dtype=bool))
        adjf_r = np.tile(a.reshape(1, GB * P * P), (HEADS, 1)).astype(f32)
        permT_r = np.zeros((NB, B), f32)
        for n in range(NB):
            permT_r[n, (r * NB + n) % B] = 1.0
        in_maps.append({
            "featT": featT_r,
            "wsl": wsl_r,
            "attp": attp_np,
            "adjf": adjf_r,
            "bnsc": bnsc_np,
            "identb": ident_np,
            "permT": permT_r.astype(bf16),
            "sel4": sel4_np,
        })
    return in_maps


def _run(inputs, trace=False):
    from concourse.bass_utils import run_bass_kernel_spmd

    if "nc" not in _NC_CACHE:
        _NC_CACHE["nc"] = _build()
    nc = _NC_CACHE["nc"]
    in_maps = _prep_inputs(**inputs)
    res = run_bass_kernel_spmd(
        nc, in_maps, core_ids=list(range(M)), trace=trace
    )
    return res


def kernel(**inputs):
    res = _run(inputs, trace=False)
    return np.asarray(res.results[0]["out"], np.float32)


# revision 33
# speedup vs baseline: 1.1025x; 1.0315x over previous
"""Trainium2 Bass kernel for nn_EmbeddingGATHead (gnn_message_passing).

Sharding (8 cores), bf16/fp16 pipelined:
  - Pooling: node-sharded. Core r owns graph nodes 24r..24r+23; streams its
    12.6 MB fp16 feature slice [2048, 24, 128] on two engine DMA queues
    (features before weights to avoid head-of-line blocking) and avg-pools
    on the vector engine -> pool_sum [128, 16kc, 24].
  - A tiny warmup AllGather issued at t=0 absorbs the ~70us first-collective
    cold cost under the pooling phase.
  - Pool AllGather (bf16), then GAT projections column-sharded by
    (proj, head) with bf16 weights (2.1 MB/layer/core). Each layer's
    AllToAll is split into two dhead-halves so the first half's trigger
    latency hides under the second half's projection matmuls.
  - Attention per core on its 24 nodes (4 cliques of 6): single batched
    z/lrelu over all heads via a merged (dc g) axis, all 4 heads' scores
    accumulated into one [4, 144] PSUM tile via zero-padded att columns,
    batched softmax, per-head alpha-broadcast matmul + weighted sum.
  - Final: residual + BN-scale folded per node, transpose + per-core
    permutation matmul places each node into its image column, one 256 KB
    AllReduce produces the [32, 2048] output everywhere.
"""
import numpy as np

B, P, C, HWF = 32, 6, 2048, 128
N = B * P            # 192
M = 8                # cores
NB = N // M          # 24 nodes/core
GB = NB // P         # 4 cliques/core
HEADS, DHEAD, LAYERS = 4, 512, 2
KCH = C // 128       # 16 contraction chunks
DC = DHEAD // 128    # 4 dhead chunks
NG = 4               # pool AllGather channel groups
KPG = KCH // NG      # kc chunks per group
GC = C // NG         # channels per group
GPP = GB * P * P     # 144 pair slots per head

_NC_CACHE = {}


def _install_drain_patch():
    """This compiler build lowers Drain to a CTRL opcode with no sync-wait
    struct; re-emit the final drain's aggregated sem waits as standalone
    wait instructions on the sync engine."""
    import bass_rust
    from concourse.vector_clock import ScopedClock
    from concourse import tile as _tile

    if getattr(_tile.TileContext, "_dab_patched", False):
        return

    def _patched_dab(self, tick_clock, wait_clock):
        nc = self.nc
        drain_inst = nc.sync.drain()
        wait_clock.add_sem_waits(
            drain_inst.ins, ScopedClock({None: tick_clock.global_clock})
        )
        si = drain_inst.ins.sync_info
        waits = list(si.on_wait) if si and si.on_wait else []
        if waits:
            si.on_wait = []
            for w in waits:
                sem = bass_rust.SemaphoreHandle(w.ant_name, w.id)
                nc.sync.wait_ge(sem, w.wait_value)
        nc.all_engine_barrier()
        popped = nc._tile_sem_poison_stack.pop()
        assert popped is self._sem_poison
        nc.clear_and_free_semaphores(list(self.sems.allocated().values()))
        nc.all_engine_barrier()

    _tile.TileContext._drain_and_barrier = _patched_dab
    _tile.TileContext._dab_patched = True


def _split_sync_waits(nc, max_waits=1):
    """This walrus build rejects instructions carrying more than one sync
    wait; hoist extras into standalone EventSemaphore waits just before the
    instruction on the same engine stream."""
    import concourse.mybir as mybir
    import bass_rust

    n = 0
    for fn in nc.m.functions:
        for bb in fn.blocks:
            insts = list(bb.instructions)
            out = []
            changed = False
            for inst in insts:
                si = inst.sync_info
                waits = list(si.on_wait) if si and si.on_wait else []
                if len(waits) > max_waits:
                    si.on_wait = waits[:max_waits]
                    for w in waits[max_waits:]:
                        n += 1
                        wi = mybir.InstEventSemaphore(
                            name=f"WSPLIT-{n}", ins=[], outs=[]
                        )
                        wi.engine = inst.engine
                        wi.sync_info = bass_rust.SyncInfo(on_wait=[w], on_update=[])
                        out.append(wi)
                    changed = True
                out.append(inst)
            if changed:
                bb.instructions = out
    return nc


def _build():
    import concourse.bass as bass
    import concourse.mybir as mybir
    from concourse import tile

    _install_drain_patch()
    dt = mybir.dt.float32
    bt = mybir.dt.bfloat16
    ft16 = mybir.dt.float16
    AF = mybir.ActivationFunctionType
    ALU = mybir.AluOpType
    AX = mybir.AxisListType
    RG = [list(range(M))]

    nc = bass.Bass(num_devices=M)

    featT = nc.declare_dram_parameter("featT", [C, NB, HWF], mybir.dt.float16, isOutput=False)
    wsl = nc.declare_dram_parameter("wsl", [LAYERS, C, DHEAD], bt, isOutput=False)
    attp = nc.declare_dram_parameter(
        "attp", [128, LAYERS * HEADS * DC, HEADS], bt, isOutput=False
    )
    adjf = nc.declare_dram_parameter("adjf", [HEADS, GPP], dt, isOutput=False)
    bnsc = nc.declare_dram_parameter("bnsc", [128, KCH, 2], dt, isOutput=False)
    identb = nc.declare_dram_parameter("identb", [128, 128], bt, isOutput=False)
    permT = nc.declare_dram_parameter("permT", [NB, B], bt, isOutput=False)
    sel4 = nc.declare_dram_parameter("sel4", [HEADS, HEADS * 128], bt, isOutput=False)
    out_ext = nc.declare_dram_parameter("out", [B, C], dt, isOutput=True)

    with tile.TileContext(nc) as tc:
        with (
            tc.tile_pool(name="dram", bufs=1, space="DRAM") as dram,
            tc.tile_pool(name="consts", bufs=1) as consts,
            tc.tile_pool(name="wpool", bufs=1) as wpool,
            tc.tile_pool(name="fpool", bufs=6) as fpool,
            tc.tile_pool(name="ppool", bufs=1) as ppool,
            tc.tile_pool(name="rpool", bufs=2) as rpool,
            tc.tile_pool(name="apool", bufs=2) as apool,
            tc.tile_pool(name="zpool", bufs=4) as zpool,
            tc.tile_pool(name="spool", bufs=2) as spool,
            tc.tile_pool(name="opool", bufs=1) as opool,
            tc.tile_pool(name="mmps", bufs=1, space="PSUM") as mmps,
            tc.tile_pool(name="sps", bufs=1, space="PSUM") as sps,
            tc.tile_pool(name="sps2", bufs=2, space="PSUM") as sps2,
        ):
            # ---------------- internal DRAM ----------------
            warm_in = dram.tile([1, B], dt)
            warm_out = dram.tile([M, B], dt, addr_space="Shared")
            ag_in = dram.tile([C, NB], bt, name="agi", tag="agi")
            ag_out = dram.tile([M, C, NB], bt, name="ago", tag="ago", addr_space="Shared")
            a2a_in = [[dram.tile([M, DHEAD // 2, NB], bt, name=f"a2ai{l}{hf}", tag=f"a2ai{l}{hf}") for hf in range(2)] for l in range(LAYERS)]
            a2a_out = [[dram.tile([M, DHEAD // 2, NB], bt, name=f"a2ao{l}{hf}", tag=f"a2ao{l}{hf}") for hf in range(2)] for l in range(LAYERS)]
            agx_in = dram.tile([C, NB], bt, name="agxi", tag="agxi")
            agx_out = dram.tile([M, C, NB], bt, name="agxo", tag="agxo", addr_space="Shared")
            ar_in = dram.tile([B, C], dt, name="ari", tag="ari")
            ar_out = dram.tile([B, C], dt, name="aro", tag="aro", addr_space="Shared")

            # ---------------- warmup collective (absorbs ncfw cold start) --
            wt = consts.tile([1, B], dt)
            nc.vector.memset(wt[:], 0.0)
            nc.sync.dma_start(warm_in[:], wt[:])
            nc.gpsimd.collective_compute(
                "AllGather", ALU.bypass, replica_groups=RG,
                ins=[warm_in.opt()], outs=[warm_out.opt()],
            )

            # ---------------- weights (bf16; loaded after feature stream) --
            w_sb = [wpool.tile([128, KCH, DHEAD], bt, name=f"w{l}", tag=f"w{l}") for l in range(LAYERS)]

            # ---------------- pooling (features on 2 DMA queues) -----------
            pool_sum = ppool.tile([128, KCH, NB], dt)
            poolb = ppool.tile([128, KCH, NB], bt)
            fview = featT.rearrange("(kc k) n w -> kc k n w", k=128)
            rts = []
            for kc in range(KCH):
                ft = fpool.tile([128, NB, HWF], ft16, tag="ft")
                eng = nc.sync if kc % 2 == 0 else nc.scalar
                eng.dma_start(ft[:], fview[kc])
                nc.vector.reduce_sum(pool_sum[:, kc, :], ft[:], axis=AX.X)
            for l in range(LAYERS):
                nc.scalar.dma_start(
                    w_sb[l][:], wsl[l].rearrange("(kc k) m -> k kc m", k=128)
                )
            # ---------------- constants ----------------
            att_sb = consts.tile([128, LAYERS * HEADS * DC, HEADS], bt)
            nc.scalar.dma_start(att_sb[:], attp[:])
            adjf_sb = consts.tile([HEADS, GPP], dt)
            nc.scalar.dma_start(adjf_sb[:], adjf[:])
            bnsc_sb = consts.tile([128, KCH, 2], dt)
            nc.scalar.dma_start(bnsc_sb[:], bnsc[:])
            ident_sb = consts.tile([128, 128], bt)
            nc.scalar.dma_start(ident_sb[:], identb[:])
            perm_sb = consts.tile([NB, B], bt)
            nc.scalar.dma_start(perm_sb[:], permT[:])
            sel_sb = consts.tile([HEADS, HEADS * 128], bt)
            nc.scalar.dma_start(sel_sb[:], sel4[:])

            nc.vector.tensor_scalar_mul(
                poolb.rearrange("p kc n -> p (kc n)"),
                pool_sum.rearrange("p kc n -> p (kc n)"), 1.0 / HWF
            )
            nc.gpsimd.dma_start(
                ag_in.rearrange("(kc k) n -> k kc n", k=128), poolb[:]
            )
            nc.gpsimd.collective_compute(
                "AllGather", ALU.bypass, replica_groups=RG,
                ins=[ag_in.opt()], outs=[ag_out.opt()],
            )

            x2_all = None
            for l in range(LAYERS):
                # ---- projections: xl^T/xr^T [512, 192] = W^T @ x^T (bf16) --
                rt = rpool.tile([128, KCH, N], bt, tag=f"rt{l}")
                src_dram = ag_out if l == 0 else agx_out
                agxv = src_dram.rearrange("r (kc k) n -> kc k r n", k=128)
                for kc in range(KCH):
                    eng = nc.sync if kc % 2 == 0 else nc.scalar
                    eng.dma_start(
                        rt[:, kc, :].rearrange("p (r n) -> p r n", r=M),
                        agxv[kc],
                    )
                pss = rpool.tile([128, DC, N], bt, tag=f"pss{l}")
                for half in range(2):
                    ps = [mmps.tile([128, N], dt, tag=f"mm{j}", name=f"mm{l}{half}{j}") for j in range(2)]
                    for j in range(2):
                        dc = half * 2 + j
                        for kc in range(KCH):
                            nc.tensor.matmul(
                                ps[j][:],
                                w_sb[l][:, kc, dc * 128:(dc + 1) * 128],
                                rt[:, kc, :],
                                start=(kc == 0),
                                stop=(kc == KCH - 1),
                            )
                    for j in range(2):
                        nc.vector.tensor_copy(pss[:, half * 2 + j, :], ps[j][:])
                    # ship this dc-half while the other half's matmuls run
                    hv = a2a_in[l][half].rearrange("s (dc d) n -> dc d s n", d=128)
                    for j in range(2):
                        nc.sync.dma_start(
                            hv[j],
                            pss[:, half * 2 + j, :].rearrange("p (r n) -> p r n", r=M),
                        )
                    nc.gpsimd.collective_compute(
                        "AllToAll", ALU.bypass, replica_groups=RG,
                        ins=[a2a_in[l][half].opt()], outs=[a2a_out[l][half].opt()],
                    )
                # ---- local attention on my 24 nodes ----
                xt = apool.tile([128, 2 * HEADS, DC, NB], bt, tag=f"xt{l}")
                for s in range(2 * HEADS):
                    eng = nc.sync if s % 2 == 0 else nc.scalar
                    for hf in range(2):
                        eng.dma_start(
                            xt[:, s, hf * 2:(hf + 1) * 2],
                            a2a_out[l][hf][s].rearrange("(dc d) n -> d dc n", d=128),
                        )
                s4 = sps.tile([HEADS, GPP], dt, tag="s4", name=f"s4{l}")
                xlv = xt[:, 0:HEADS].rearrange(
                    "p h dc (g i) -> p h (dc g) i", g=GB
                )[:, :, :, None, :].to_broadcast([128, HEADS, DC * GB, P, P])
                xrv = xt[:, HEADS:2 * HEADS].rearrange(
                    "p h dc (g i) -> p h (dc g) i", g=GB
                )[:, :, :, :, None].to_broadcast([128, HEADS, DC * GB, P, P])
                z2 = zpool.tile([128, HEADS, DC * GB, P, P], bt, tag="z")
                nc.vector.tensor_tensor(z2[:], xrv, xlv, ALU.add)
                zf = z2.rearrange("p h dcg i j -> p (h dcg i j)")
                lz = zpool.tile([128, HEADS * DC * GPP], bt, tag="lz")
                # lrelu(z) = max(0.2*z, z)
                nc.vector.scalar_tensor_tensor(
                    lz[:], zf, 0.2, zf, ALU.mult, ALU.max
                )
                for h in range(HEADS):
                    for dc in range(DC):
                        off = (h * KCH + dc * GB) * P * P
                        nc.tensor.matmul(
                            s4[:],
                            att_sb[:, (l * HEADS + h) * DC + dc, :],
                            lz[:, off:off + GPP],
                            start=(h == 0 and dc == 0),
                            stop=(h == HEADS - 1 and dc == DC - 1),
                        )
                # masked softmax over the 6 sources (exp without max-shift)
                e4 = spool.tile([HEADS, GPP], dt, tag=f"e4{l}", name=f"e4{l}")
                nc.scalar.activation(e4[:], s4[:], AF.Exp)
                em4 = spool.tile([HEADS, GPP], dt, tag=f"em{l}", name=f"em{l}")
                nc.vector.tensor_tensor(em4[:], e4[:], adjf_sb[:], ALU.mult)
                ssum = spool.tile([HEADS, GB * P], dt, tag=f"ss{l}", name=f"ss{l}")
                nc.vector.reduce_sum(
                    ssum[:], em4.rearrange("p (gi j) -> p gi j", j=P), axis=AX.X
                )
                rec = spool.tile([HEADS, GB * P], dt, tag=f"rc{l}", name=f"rc{l}")
                nc.vector.reciprocal(rec[:], ssum[:])
                alpha4 = spool.tile([HEADS, GPP], bt, tag=f"al{l}", name=f"al{l}")
                nc.vector.tensor_tensor(
                    alpha4.rearrange("p (gi j) -> p gi j", j=P),
                    em4.rearrange("p (gi j) -> p gi j", j=P),
                    rec[:, :, None].to_broadcast([HEADS, GB * P, P]),
                    ALU.mult,
                )
                # aggregation: out[i] = sum_j alpha[i,j] xl[j]
                out_all = opool.tile([128, HEADS, DC, NB], dt, tag=f"oa{l}")
                for h in range(HEADS):
                    ab_ps = sps.tile([128, GPP], dt, tag="ab", name=f"ab{l}{h}")
                    nc.tensor.matmul(
                        ab_ps[:], sel_sb[:, h * 128:(h + 1) * 128], alpha4[:],
                        start=True, stop=True,
                    )
                    ab = apool.tile([128, GPP], bt, tag="abs")
                    nc.vector.tensor_copy(ab[:], ab_ps[:])
                    ab5 = ab.rearrange("p (g i j) -> p g i j", g=GB, i=P)[
                        :, None, :, :, :
                    ].to_broadcast([128, DC, GB, P, P])
                    xl5 = xt[:, h].rearrange("p dc (g i) -> p dc g i", g=GB)[
                        :, :, :, None, :
                    ].to_broadcast([128, DC, GB, P, P])
                    prod = zpool.tile([128, DC, GB, P, P], bt, tag="pr")
                    nc.vector.tensor_tensor(prod[:], ab5, xl5, ALU.mult)
                    nc.vector.reduce_sum(
                        out_all[:, h],
                        prod.rearrange("p dc g i j -> p dc (g i) j"),
                        axis=AX.X,
                    )
                oflat = out_all.rearrange("p h dc n -> p (h dc n)")
                if l == 0:
                    t1 = opool.tile([128, HEADS * DC * NB], dt, tag="t1")
                    nc.vector.tensor_scalar_min(t1[:], oflat, 0.0)
                    t2 = opool.tile([128, HEADS * DC * NB], dt, tag="t2")
                    nc.scalar.activation(t2[:], t1[:], AF.Exp)
                    x1b = opool.tile([128, HEADS * DC * NB], bt, tag="x1b")
                    # elu(x) = max(exp(min(x,0)) - 1, x)
                    nc.vector.scalar_tensor_tensor(
                        x1b[:], t2[:], -1.0, oflat, ALU.add, ALU.max
                    )
                    nc.sync.dma_start(
                        agx_in.rearrange("(hd d) n -> d hd n", d=128),
                        x1b.rearrange("p (hd n) -> p hd n", n=NB),
                    )
                    nc.gpsimd.collective_compute(
                        "AllGather", ALU.bypass, replica_groups=RG,
                        ins=[agx_in.opt()], outs=[agx_out.opt()],
                    )
                else:
                    x2_all = opool.tile([128, KCH, NB], dt, tag="x2")
                    # residual: out + pool_sum/128
                    nc.vector.scalar_tensor_tensor(
                        x2_all.rearrange("p kc n -> p (kc n)"),
                        pool_sum.rearrange("p kc n -> p (kc n)"),
                        1.0 / HWF, oflat, ALU.mult, ALU.add,
                    )

            # ------- final: BN-scale, transpose, node->image perm, AllReduce
            arin = ppool.tile([B, KCH, 128], dt)
            for kc in range(KCH):
                x2s = rpool.tile([128, NB], bt, tag="x2s")
                nc.scalar.activation(
                    x2s[:], x2_all[:, kc, :], AF.Identity,
                    bias=bnsc_sb[:, kc, 1:2], scale=bnsc_sb[:, kc, 0:1],
                )
                tp = sps2.tile([NB, 128], bt, tag="tp")
                nc.tensor.transpose(tp[:], x2s[:], ident_sb[:])
                x2t = rpool.tile([NB, 128], bt, tag="x2t")
                nc.vector.tensor_copy(x2t[:], tp[:])
                arps = sps2.tile([B, 128], dt, tag="arps")
                nc.tensor.matmul(arps[:], perm_sb[:], x2t[:], start=True, stop=True)
                nc.scalar.copy(arin[:, kc, :], arps[:])
            nc.sync.dma_start(
                ar_in.rearrange("b (kc d) -> b kc d", d=128), arin[:]
            )
            nc.gpsimd.collective_compute(
                "AllReduce", ALU.add, replica_groups=RG,
                ins=[ar_in.opt()], outs=[ar_out.opt()],
            )
            nc.sync.dma_start(out_ext[:], ar_out[:])

    _split_sync_waits(nc)
    return nc


def _prep_inputs(features, img_num_ps, Wl, bl, Wr, br, att, gat_bias,
                 bn_gamma, bn_mean, bn_var):
    import ml_dtypes
    f32 = np.float32
    bf16 = ml_dtypes.bfloat16
    features = np.asarray(features, f32)
    inp = np.asarray(img_num_ps)
    Wl = np.asarray(Wl, f32)
    Wr = np.asarray(Wr, f32)
    att = np.asarray(att, f32)
    bn_gamma = np.asarray(bn_gamma, f32)
    bn_mean = np.asarray(bn_mean, f32)
    bn_var = np.asarray(bn_var, f32)

    parts = features.reshape(B, P, C, HWF).transpose(1, 0, 2, 3).reshape(N, C, HWF)
    # zero-padded per-head att columns: attp[(l,h,dc), k, m] = att[l,h,dc*128+k]
    # iff m == h else 0
    attp_np = np.zeros((LAYERS, HEADS, DC, 128, HEADS), f32)
    for l in range(LAYERS):
        for h in range(HEADS):
            attp_np[l, h, :, :, h] = att[l, h].reshape(DC, 128)
    attp_np = np.ascontiguousarray(
        attp_np.reshape(LAYERS * HEADS * DC, 128, HEADS).transpose(1, 0, 2)
    ).astype(bf16)
    scale = bn_gamma / np.sqrt(bn_var + 1e-5)
    bnsc_np = np.ascontiguousarray(np.stack(
        [(scale / P).reshape(KCH, 128),
         (-scale * bn_mean / P).reshape(KCH, 128)],
        axis=1,
    ).transpose(2, 0, 1)).astype(f32)
    ident_np = np.eye(128, dtype=f32).astype(bf16)
    sel4_np = np.zeros((HEADS, HEADS * 128), f32)
    for h in range(HEADS):
        sel4_np[h, h * 128:(h + 1) * 128] = 1.0
    sel4_np = sel4_np.astype(bf16)

    in_maps = []
    for r in range(M):
        featT_r = np.ascontiguousarray(
            parts[r * NB:(r + 1) * NB].transpose(1, 0, 2)
        ).astype(np.float16)
        wsl_r = np.ascontiguousarray(
            (Wl if r < HEADS else Wr)[:, r % HEADS]
        ).astype(bf16)
        a = np.zeros((GB, P, P), f32)
        for gl in range(GB):
            v = np.arange(P) < inp[GB * r + gl]
            a[gl] = ((v[:, None] & v[None, :]) | np.eye(P, dtype=bool))
        adjf_r = np.tile(a.reshape(1, GB * P * P), (HEADS, 1)).astype(f32)
        permT_r = np.zeros((NB, B), f32)
        for n in range(NB):
            permT_r[n, (r * NB + n) % B] = 1.0
        in_maps.append({
            "featT": featT_r,
            "wsl": wsl_r,
            "attp": attp_np,
            "adjf": adjf_r,
            "bnsc": bnsc_np,
            "identb": ident_np,
            "permT": permT_r.astype(bf16),
            "sel4": sel4_np,
        })
    return in_maps


def _run(inputs, trace=False):
    from concourse.bass_utils import run_bass_kernel_spmd

    if "nc" not in _NC_CACHE:
        _NC_CACHE["nc"] = _build()
    nc = _NC_CACHE["nc"]
    in_maps = _prep_inputs(**inputs)
    res = run_bass_kernel_spmd(
        nc, in_maps, core_ids=list(range(M)), trace=trace
    )
    return res


def kernel(**inputs):
    res = _run(inputs, trace=False)
    return np.asarray(res.results[0]["out"], np.float32)
